# revision 2
# baseline (speedup 1.0000x reference)
"""Trainium2 kernel for nn_NonsharedPatchEmbed: 196 independent Linear(768->768)
applied per patch of a patchified [64, 3, 224, 224] image batch.

  out[b, p, o] = sum_i patches[b, p, i] * W[p, o, i] + b[p, o]

Strategy: shard the 196-patch axis across the 8 NeuronCores (25 patches per
core, padded to 200). Per patch this is a [64, 768] x [768, 768]^T GEMM with
the full batch as the stationary operand; the per-patch W (the dominant
traffic, 462 MB total) streams through the tensor engine exactly once.

Host-side work is layout only: patchify x, pre-transpose W to W^T, and split
the bias into a bf16 hi+lo pair (added exactly via a K=2 ones-matmul into the
same PSUM accumulation group).

Layouts per core (mode A, W moving):
  aT  [128, 25, 6, 64]  f32   aT[i, p, c, b] = patches[b, 25k+p, 128c+i]
  Wt  [25, 128, 6, 768] f32   Wt[p, i, c, o] = W[25k+p, o, 128c+i]
  bhl [2, 25, 768]      bf16  bias hi/lo split
  out [25, 64, 768]     f32

Mode B (W stationary): same inputs plus bias_pp [128, 25, 6] f32; per
(patch, o_chunk) accumulates psum [128, 64] over 6 i-chunks with W^T blocks
as lhsT; bias added per-partition during the PSUM->SBUF copy.
"""

import os
import numpy as np
import ml_dtypes

import concourse.bass as bass
import concourse.tile as tile
import concourse.mybir as mybir
from concourse import bacc
from concourse.bass_utils import run_bass_kernel_spmd

f32 = mybir.dt.float32
bf16 = mybir.dt.bfloat16

N_CORES = 8
B = 64
D = 768
NP = 196
PPC = 25          # patches per core (8*25 = 200, last 4 padded)
NCHUNK = 6        # 768 / 128

LAST_RESULTS = None  # BassKernelResults of the most recent run (for test.py)

_NC_CACHE = {}


def _build(mode):
    nc = bacc.Bacc()
    aT = nc.declare_dram_parameter("aT", [128, PPC, NCHUNK, B], f32, isOutput=False)
    Wt = nc.declare_dram_parameter("Wt", [PPC, 128, NCHUNK, D], f32, isOutput=False)
    if mode == "A":
        bhl = nc.declare_dram_parameter("bhl", [2, PPC, D], bf16, isOutput=False)
        out = nc.declare_dram_parameter("out", [PPC, B, D], f32, isOutput=True)
    else:
        bpp = nc.declare_dram_parameter("bpp", [128, PPC, NCHUNK], f32, isOutput=False)
        out = nc.declare_dram_parameter("out", [PPC, 128, NCHUNK, B], f32, isOutput=True)

    with tile.TileContext(nc) as tc:
        with (
            tc.tile_pool(name="const", bufs=1) as cpool,
            tc.tile_pool(name="w", bufs=3) as wpool,
            tc.tile_pool(name="o", bufs=3) as opool,
            tc.tile_pool(name="ps", bufs=4, space="PSUM") as pspool,
        ):
            ta = cpool.tile([128, PPC, NCHUNK, B], f32)
            nc.sync.dma_start(ta[:], aT[:])
            if mode == "A":
                ones = cpool.tile([2, B], bf16)
                nc.vector.memset(ones[:], 1.0)
                tb = cpool.tile([2, PPC, D], bf16)
                nc.sync.dma_start(tb[:], bhl[:])
            else:
                tbias = cpool.tile([128, PPC, NCHUNK], f32)
                nc.sync.dma_start(tbias[:], bpp[:])

            for p in range(PPC):
                wt = wpool.tile([128, NCHUNK, D], f32)
                nc.sync.dma_start(wt[:], Wt[p])

                if mode == "A":
                    pt = pspool.tile([B, D], f32)
                    slices = [(0, 512), (512, 768)]
                    for (o0, o1) in slices:
                        nc.tensor.matmul(
                            pt[:, o0:o1], ones[:], tb[:, p, o0:o1],
                            start=True, stop=False,
                        )
                    for c in range(NCHUNK):
                        for (o0, o1) in slices:
                            nc.tensor.matmul(
                                pt[:, o0:o1], ta[:, p, c, :], wt[:, c, o0:o1],
                                start=False, stop=(c == NCHUNK - 1),
                            )
                    ob = opool.tile([B, D], f32)
                    nc.vector.tensor_copy(ob[:], pt[:])
                    nc.sync.dma_start(out[p], ob[:])
                else:
                    ob = opool.tile([128, NCHUNK, B], f32)
                    for oc in range(NCHUNK):
                        pt = pspool.tile([128, B], f32)
                        for c in range(NCHUNK):
                            nc.tensor.matmul(
                                pt[:], wt[:, c, oc * 128:(oc + 1) * 128],
                                ta[:, p, c, :],
                                start=(c == 0), stop=(c == NCHUNK - 1),
                            )
                        nc.vector.tensor_scalar_add(
                            ob[:, oc, :], pt[:], tbias[:, p, oc:oc + 1]
                        )
                    nc.sync.dma_start(out[p], ob[:])

    nc.finalize()
    return nc


def _patchify(x):
    # [B, C, H, W] -> [B, 196, 768] in MAE ordering (n c h p w q -> n h w p q c)
    Bn, C, H, Wd = x.shape
    h = H // 16
    xr = x.reshape(Bn, C, h, 16, h, 16)
    xr = np.transpose(xr, (0, 2, 4, 3, 5, 1))
    return xr.reshape(Bn, h * h, 16 * 16 * C)


def kernel(x, W, b, _trace=False, _mode=None):
    global LAST_RESULTS
    mode = _mode or os.environ.get("KERNEL_MODE", "A")

    x = np.asarray(x, dtype=np.float32)
    W = np.asarray(W, dtype=np.float32)
    b = np.asarray(b, dtype=np.float32)

    patches = _patchify(x)                      # [64, 196, 768]

    in_maps = []
    for k in range(N_CORES):
        lo = k * PPC
        idx = np.arange(lo, lo + PPC)
        idx[idx >= NP] = 0                      # pad tail with patch 0
        psl = patches[:, idx, :]                # [64, 25, 768]
        wsl = W[idx]                            # [25, 768, 768]
        bsl = b[idx]                            # [25, 768]

        aT = np.ascontiguousarray(
            psl.transpose(2, 1, 0)              # [768, 25, 64]
            .reshape(NCHUNK, 128, PPC, B)
            .transpose(1, 2, 0, 3)              # [128, 25, 6, 64]
        )
        Wt = np.ascontiguousarray(
            wsl.transpose(0, 2, 1)              # [25, 768(i), 768(o)]
            .reshape(PPC, NCHUNK, 128, D)
            .transpose(0, 2, 1, 3)              # [25, 128, 6, 768]
        )
        m = {"aT": aT, "Wt": Wt}
        if mode == "A":
            hi = bsl.astype(ml_dtypes.bfloat16)
            lo_ = (bsl - hi.astype(np.float32)).astype(ml_dtypes.bfloat16)
            m["bhl"] = np.ascontiguousarray(np.stack([hi, lo_], axis=0))
        else:
            m["bpp"] = np.ascontiguousarray(
                bsl.reshape(PPC, NCHUNK, 128).transpose(2, 0, 1)
            )
        in_maps.append(m)

    key = mode
    if key not in _NC_CACHE:
        _NC_CACHE[key] = _build(mode)
    nc = _NC_CACHE[key]

    res = run_bass_kernel_spmd(nc, in_maps, list(range(N_CORES)), trace=_trace)
    LAST_RESULTS = res

    parts = np.stack([res.results[k]["out"] for k in range(N_CORES)])
    if mode == "A":
        # parts [8, 25, 64, 768] -> [64, 200, 768]
        full = parts.transpose(2, 0, 1, 3).reshape(B, N_CORES * PPC, D)
    else:
        # parts [8, 25, 128(o_in), 6(oc), 64(b)] -> [64, 200, 768]
        full = parts.transpose(4, 0, 1, 3, 2).reshape(B, N_CORES * PPC, D)
    return np.ascontiguousarray(full[:, :NP, :])
